# revision 20
# baseline (speedup 1.0000x reference)
# Trainium2 Bass kernel for nn_CounterfactualGenerator (gnn_message_passing).
#
# Strategy:
#   * Pure data parallel over the batch dim B across 8 NeuronCores.
#   * The sequential 3x2048-edge propagation scan is a LINEAR operator on each
#     row's V-vector: every edge update d[:,e] += d[:,c]*(s*0.1) is an
#     elementary matrix A_i = I + coef*E_{c,e}; the full scan is d0 @ (A_1...A_N).
#     We form M = prod A_i on the host in float64 (O(E*V) work, ~ms) and apply
#     it on-device as one [B,V]x[V,V] matmul.  To keep fp32 accuracy with bf16
#     matmuls we split M = I + Mc and compute cf = pre + pre@Mc (the identity
#     part stays exact fp32; only the correction sees bf16 rounding).
#   * Everything on device lives in "transposed" layout [feature, batch]:
#     all matmuls contract over SBUF partitions, batch streams along the free
#     dim (N=512 chunks, one PSUM bank per matmul).
#   * The plausibility input concat [x, intervened, cf] is algebraically
#     folded: intervened = x + delta (delta nonzero on the K intervened rows
#     only), so  (pl_W1x + pl_W1int)^T x + pl_W1int[ivars]^T delta
#     + pl_W1cf^T cf, with the combine done on the host and the delta term a
#     K=4 matmul. 'intervened' (bf16) is never materialized on device; the
#     fp32 intervened matrix (needed for pre = intervened + eff) is built on
#     the host and shipped instead of fp32 x.
#   * Host does: input transpose/shard, M precompute, final transpose back,
#     sigmoid (plaus) and the row-norm (impact) from returned final_cf.
import sys

sys.path.insert(0, "/opt/trn_rl_repo")

import numpy as np
import ml_dtypes

import concourse.bass as bass
import concourse.mybir as mybir
import concourse.tile as tile
from concourse import bacc
from concourse.bass_utils import run_bass_kernel_spmd

NCORES = 8
V = 256
K = 4
NB = 512  # matmul free-dim chunk (one PSUM bank)
PROP_SCALE = 0.1
INTERVENTION_STRENGTH = 1.0

F32 = mybir.dt.float32
BF16 = mybir.dt.bfloat16
bf16 = ml_dtypes.bfloat16

_KERNEL_CACHE = {}


def _build_nc(Bc, has_b1, has_b2, has_pb1, has_pb2):
    """Build the per-core Bass module. Bc = batch elements per core."""
    nchunk = Bc // NB
    nc = bacc.Bacc("TRN2", target_bir_lowering=False, debug=False, num_devices=NCORES)

    # ---- DRAM I/O ----
    inT = nc.dram_tensor("inT", [V, Bc], F32, kind="ExternalInput")  # intervened^T
    xTb = nc.dram_tensor("xTb", [V, Bc], BF16, kind="ExternalInput")  # x^T bf16
    # [0:K] = |iv - x_iv| (mag), [K:2K] = iv - x_iv (delta for plaus fold)
    smb = nc.dram_tensor("smb", [2 * K, Bc], BF16, kind="ExternalInput")
    w1t = nc.dram_tensor("W1T", [V, V], BF16, kind="ExternalInput")  # [vin, o]
    w2t = nc.dram_tensor("W2T", [V, V], BF16, kind="ExternalInput")  # [o, vout]
    mct = nc.dram_tensor("MC", [V, V], BF16, kind="ExternalInput")  # [vin, vout]
    selt = nc.dram_tensor("SEL", [K, V], BF16, kind="ExternalInput")
    pw1xt = nc.dram_tensor("PW1X", [V, 128], BF16, kind="ExternalInput")
    pw1ct = nc.dram_tensor("PW1C", [V, 128], BF16, kind="ExternalInput")
    pw1dt = nc.dram_tensor("PW1D", [K, 128], BF16, kind="ExternalInput")
    pw2t = nc.dram_tensor("PW2T", [128, 64], BF16, kind="ExternalInput")
    pw3t = nc.dram_tensor("PW3T", [64, 1], BF16, kind="ExternalInput")
    if has_b1:
        b1t = nc.dram_tensor("B1", [V, 1], F32, kind="ExternalInput")
    if has_b2:
        b2t = nc.dram_tensor("B2", [K, V], BF16, kind="ExternalInput")
    if has_pb1:
        pb1t = nc.dram_tensor("PB1", [128, 1], F32, kind="ExternalInput")
    if has_pb2:
        pb2t = nc.dram_tensor("PB2", [64, 1], F32, kind="ExternalInput")

    outC = nc.dram_tensor("outC", [V, Bc], F32, kind="ExternalOutput")  # final_cf^T
    outE = nc.dram_tensor("outE", [V, Bc], F32, kind="ExternalOutput")  # effects^T
    outP = nc.dram_tensor("outP", [1, Bc], F32, kind="ExternalOutput")  # plaus logits

    # chunk-pair views: [part, vchunk, batch]
    inTv = inT.rearrange("(c p) b -> p c b", p=128)
    xTbv = xTb.rearrange("(c p) b -> p c b", p=128)
    outCv = outC.rearrange("(c p) b -> p c b", p=128)
    outEv = outE.rearrange("(c p) b -> p c b", p=128)

    AF = mybir.ActivationFunctionType

    with tile.TileContext(nc) as tc:
        with (
            tc.tile_pool(name="consts", bufs=1) as consts,
            tc.tile_pool(name="work", bufs=4) as work,
            tc.tile_pool(name="psum", bufs=1, space="PSUM") as psum,
        ):
            # ---- load replicated weights once ----
            w1 = [consts.tile([128, V], BF16, name=f"w1_{c}") for c in range(2)]
            w2 = [consts.tile([128, V], BF16, name=f"w2_{c}") for c in range(2)]
            mc = [consts.tile([128, V], BF16, name=f"mc_{c}") for c in range(2)]
            for c in range(2):
                nc.sync.dma_start(out=w1[c], in_=w1t[c * 128 : (c + 1) * 128, :])
                nc.sync.dma_start(out=w2[c], in_=w2t[c * 128 : (c + 1) * 128, :])
                nc.sync.dma_start(out=mc[c], in_=mct[c * 128 : (c + 1) * 128, :])
            pw1x = [consts.tile([128, 128], BF16, name=f"pw1x_{c}") for c in range(2)]
            pw1c = [consts.tile([128, 128], BF16, name=f"pw1c_{c}") for c in range(2)]
            for c in range(2):
                nc.sync.dma_start(out=pw1x[c], in_=pw1xt[c * 128 : (c + 1) * 128, :])
                nc.sync.dma_start(out=pw1c[c], in_=pw1ct[c * 128 : (c + 1) * 128, :])
            sel = consts.tile([K, V], BF16, name="sel")
            nc.sync.dma_start(out=sel, in_=selt[:, :])
            pw1d = consts.tile([K, 128], BF16, name="pw1d")
            nc.sync.dma_start(out=pw1d, in_=pw1dt[:, :])
            pw2 = consts.tile([128, 64], BF16, name="pw2")
            nc.sync.dma_start(out=pw2, in_=pw2t[:, :])
            pw3 = consts.tile([64, 1], BF16, name="pw3")
            nc.sync.dma_start(out=pw3, in_=pw3t[:, :])
            b1 = None
            if has_b1:
                b1 = [consts.tile([128, 1], F32, name=f"b1_{c}") for c in range(2)]
                for c in range(2):
                    nc.sync.dma_start(out=b1[c], in_=b1t[c * 128 : (c + 1) * 128, :])
            b2 = None
            if has_b2:
                b2 = consts.tile([K, V], BF16, name="b2")
                nc.sync.dma_start(out=b2, in_=b2t[:, :])
            pb1 = None
            if has_pb1:
                pb1 = consts.tile([128, 1], F32, name="pb1")
                nc.sync.dma_start(out=pb1, in_=pb1t[:, :])
            pb2 = None
            if has_pb2:
                pb2 = consts.tile([64, 1], F32, name="pb2")
                nc.sync.dma_start(out=pb2, in_=pb2t[:, :])

            # ---- hoisted mag broadcast: mf_all[p, c, b] = mag[(c*128+p)//64, b]
            # via a single DMA with a partition-broadcast access pattern
            dvall = consts.tile([K, Bc], BF16, name="dvall")
            nc.sync.dma_start(out=dvall, in_=smb[K : 2 * K, :])
            xball = consts.tile([128, 2, Bc], BF16, name="xball")
            nc.sync.dma_start(out=xball, in_=xTbv[:, :, :])
            plfall = consts.tile([1, Bc], F32, name="plfall")
            mf_all = consts.tile([128, 2, Bc], BF16, name="mf_all")
            magall = consts.tile([K, Bc], BF16, name="magall")
            nc.sync.dma_start(out=magall, in_=smb[0:K, :])
            for j in range(nchunk):
                colsj = slice(j * NB, (j + 1) * NB)
                for c in range(2):
                    mfp = psum.tile([128, NB], F32, name="mfp", tag="ps1", bufs=6)
                    nc.tensor.matmul(mfp, sel[:, c * 128 : (c + 1) * 128],
                                     magall[:, colsj], start=True, stop=True)
                    if c == 0:
                        nc.scalar.copy(mf_all[:, c, colsj], mfp)
                    else:
                        nc.vector.tensor_copy(mf_all[:, c, colsj], mfp)

            # ---- per-chunk pipeline ----
            for j in range(nchunk):
                cols = slice(j * NB, (j + 1) * NB)

                inf = work.tile([128, 2, NB], F32, name="inf", tag="inf")
                nc.sync.dma_start(out=inf, in_=inTv[:, :, cols])
                xb = xball[:, :, cols]
                dvb = dvall[:, cols]

                # h^T = relu(W1cat @ x + b1)  [256, NB]
                hp = [psum.tile([128, NB], F32, name=f"hp{c}", tag="ps1", bufs=6)
                      for c in range(2)]
                for c in range(2):
                    nc.tensor.matmul(hp[c], w1[0][:, c * 128 : (c + 1) * 128],
                                     xb[:, 0, :], start=True, stop=False)
                    nc.tensor.matmul(hp[c], w1[1][:, c * 128 : (c + 1) * 128],
                                     xb[:, 1, :], start=False, stop=True)
                hr = work.tile([128, 2, NB], BF16, name="hr", tag="hr")
                for c in range(2):
                    nc.scalar.activation(hr[:, c, :], hp[c], AF.Relu,
                                         bias=b1[c] if has_b1 else 0.0)
                # hs = h * mag_broadcast  (bf16 x bf16, single 2x-mode op)
                hs = work.tile([128, 2, NB], BF16, name="hs", tag="hs")
                nc.vector.tensor_mul(hs, hr, mf_all[:, :, cols])

                # eff^T = W2cat @ hs (+ b2-part)  [256, NB]
                ep = [psum.tile([128, NB], F32, name=f"ep{c}", tag="ps1", bufs=6)
                      for c in range(2)]
                for c in range(2):
                    nc.tensor.matmul(ep[c], w2[0][:, c * 128 : (c + 1) * 128],
                                     hs[:, 0, :], start=True, stop=False)
                    nc.tensor.matmul(ep[c], w2[1][:, c * 128 : (c + 1) * 128],
                                     hs[:, 1, :], start=False, stop=not has_b2)
                    if has_b2:
                        nc.tensor.matmul(ep[c], b2[:, c * 128 : (c + 1) * 128],
                                         magall[:, cols], start=False, stop=True)
                ef = work.tile([128, 2, NB], F32, name="ef", tag="ef")
                nc.scalar.copy(ef[:, 0, :], ep[0])
                nc.vector.tensor_copy(ef[:, 1, :], ep[1])
                nc.scalar.dma_start(out=outEv[:, :, cols], in_=ef)

                # pre = intervened + eff (fp32)
                pf = work.tile([128, 2, NB], F32, name="pf", tag="pf")
                nc.vector.tensor_add(pf, ef, inf)
                pb = work.tile([128, 2, NB], BF16, name="pb", tag="pb")
                nc.vector.tensor_copy(pb, pf)

                # cf = pre + pre @ Mc
                cp = [psum.tile([128, NB], F32, name=f"cp{c}", tag="ps1", bufs=6)
                      for c in range(2)]
                for c in range(2):
                    nc.tensor.matmul(cp[c], mc[0][:, c * 128 : (c + 1) * 128],
                                     pb[:, 0, :], start=True, stop=False)
                    nc.tensor.matmul(cp[c], mc[1][:, c * 128 : (c + 1) * 128],
                                     pb[:, 1, :], start=False, stop=True)
                cf = work.tile([128, 2, NB], F32, name="cf", tag="cf")
                for c in range(2):
                    nc.vector.tensor_add(cf[:, c, :], pf[:, c, :], cp[c])
                nc.gpsimd.dma_start(out=outCv[:, :, cols], in_=cf)

                # plausibility MLP: pin = [x, intervened, cf] folded; the cf
                # block reads pre instead via PW1C := M @ pl_W1cf^T (host):
                # p1 = (W1x+W1int)^T x + W1int[ivars]^T dv + (M@W1cf)^T pre
                p1p = psum.tile([128, NB], F32, name="p1p", tag="ps2", bufs=1)
                nc.tensor.matmul(p1p, pw1x[0], xb[:, 0, :], start=True, stop=False)
                nc.tensor.matmul(p1p, pw1x[1], xb[:, 1, :], start=False, stop=False)
                nc.tensor.matmul(p1p, pw1d, dvb, start=False, stop=False)
                nc.tensor.matmul(p1p, pw1c[0], pb[:, 0, :], start=False, stop=False)
                nc.tensor.matmul(p1p, pw1c[1], pb[:, 1, :], start=False, stop=True)
                p1b = work.tile([128, NB], BF16, name="p1b", tag="p1b")
                nc.scalar.activation(p1b, p1p, AF.Relu,
                                     bias=pb1 if has_pb1 else 0.0)
                p2p = psum.tile([64, NB], F32, name="p2p", tag="ps2", bufs=1)
                nc.tensor.matmul(p2p, pw2, p1b, start=True, stop=True)
                p2b = work.tile([64, NB], BF16, name="p2b", tag="p2b")
                nc.scalar.activation(p2b, p2p, AF.Relu,
                                     bias=pb2 if has_pb2 else 0.0)
                plp = psum.tile([1, NB], F32, name="plp", tag="ps2", bufs=1)
                nc.tensor.matmul(plp, pw3, p2b, start=True, stop=True)
                nc.scalar.copy(plfall[:, cols], plp)

                # keep-alive fillers: low-priority matmuls into a scratch bank
                # that the scheduler drops into PE gaps so the HAM clock-gate
                # stays at 8/8 (no consumer; WAW-chained through the bank)
                if j < nchunk - 1:
                    for _f in range(5):
                        fp = psum.tile([128, NB], F32, name="fp", tag="fill",
                                       bufs=1)
                        nc.tensor.matmul(fp, w1[0][:, 0:128],
                                         xball[:, 0, cols], start=True,
                                         stop=True)

            nc.sync.dma_start(out=outP[:, :], in_=plfall)

    nc.compile()
    return nc


def _prep_propagation_matrix(cause_idx, effect_idx, strengths, n_rounds=3):
    M = np.eye(V, dtype=np.float64)
    coef = strengths.astype(np.float64) * PROP_SCALE
    c = cause_idx.astype(np.int64)
    e = effect_idx.astype(np.int64)
    for _ in range(n_rounds):
        for i in range(len(c)):
            M[:, e[i]] += coef[i] * M[:, c[i]]
    return M


def kernel(original_data, intervention_values, strengths,
           est_W1, est_b1, est_W2, est_b2,
           pl_W1, pl_b1, pl_W2, pl_b2, pl_W3, pl_b3,
           intervention_variables, cause_idx, effect_idx):
    x = np.ascontiguousarray(np.asarray(original_data, dtype=np.float32))
    iv = np.asarray(intervention_values, dtype=np.float32)
    ivars = np.asarray(intervention_variables, dtype=np.int64)
    B = x.shape[0]
    assert x.shape[1] == V and iv.shape[1] == K
    assert B % (NCORES * NB) == 0, "batch must divide 8*512"
    Bc = B // NCORES

    est_b1 = np.asarray(est_b1, np.float32)
    est_b2 = np.asarray(est_b2, np.float32)
    pl_b1 = np.asarray(pl_b1, np.float32)
    pl_b2 = np.asarray(pl_b2, np.float32)
    pl_b3 = np.asarray(pl_b3, np.float32)
    has_b1 = bool(np.any(est_b1 != 0))
    has_b2 = bool(np.any(est_b2 != 0))
    has_pb1 = bool(np.any(pl_b1 != 0))
    has_pb2 = bool(np.any(pl_b2 != 0))

    # ---- host precompute ----
    M = _prep_propagation_matrix(np.asarray(cause_idx), np.asarray(effect_idx),
                                 np.asarray(strengths, np.float32))
    Mc = (M - np.eye(V)).astype(np.float32)

    W1cat = np.asarray(est_W1, np.float32).reshape(K * 64, V)  # [o, vin]
    W1T = np.ascontiguousarray(W1cat.T).astype(bf16)  # [vin, o]
    W2T = np.ascontiguousarray(
        np.asarray(est_W2, np.float32).transpose(0, 2, 1).reshape(K * 64, V)
    ).astype(bf16)  # [o, vout]
    MC = Mc.astype(bf16)  # [vin, vout]
    SEL = np.zeros((K, V), np.float32)
    for k in range(K):
        SEL[k, k * 64:(k + 1) * 64] = 1.0
    SEL = SEL.astype(bf16)
    pl_W1 = np.asarray(pl_W1, np.float32)  # [128, 3V]
    PW1X = np.ascontiguousarray((pl_W1[:, 0:V] + pl_W1[:, V:2 * V]).T).astype(bf16)
    PW1C = np.ascontiguousarray(
        M.astype(np.float64) @ pl_W1[:, 2 * V:3 * V].T.astype(np.float64)
    ).astype(np.float32).astype(bf16)
    # delta rows of the intervened block, deduped (last write wins)
    PW1D = np.zeros((K, 128), np.float32)
    last = {}
    for k, vv in enumerate(ivars.tolist()):
        last[vv] = k
    for k, vv in enumerate(ivars.tolist()):
        if last[vv] == k:
            PW1D[k] = pl_W1[:, V + vv]
    PW1D = PW1D.astype(bf16)
    PW2T = np.ascontiguousarray(np.asarray(pl_W2, np.float32).T).astype(bf16)
    PW3T = np.ascontiguousarray(np.asarray(pl_W3, np.float32).T).astype(bf16)

    intv = x.copy()
    intv[:, ivars] = iv
    inT = np.ascontiguousarray(intv.T)  # [V, B] fp32
    xTb = np.ascontiguousarray(x.T).astype(bf16)
    x_iv = x[:, ivars]  # [B, K]
    dv = iv - x_iv  # [B, K]
    # dedupe delta contributions for the plaus fold (last write wins)
    dvz = dv.copy()
    for k, vv in enumerate(ivars.tolist()):
        if last[vv] != k:
            dvz[:, k] = 0.0
    smb = np.concatenate([np.abs(dv).T, dvz.T], axis=0).astype(bf16)  # [2K, B]

    key = (Bc, has_b1, has_b2, has_pb1, has_pb2)
    if key not in _KERNEL_CACHE:
        _KERNEL_CACHE[key] = _build_nc(*key)
    nc = _KERNEL_CACHE[key]

    shared = {
        "W1T": W1T, "W2T": W2T, "MC": MC, "SEL": SEL,
        "PW1X": PW1X, "PW1C": PW1C, "PW1D": PW1D,
        "PW2T": PW2T, "PW3T": PW3T,
    }
    if has_b1:
        shared["B1"] = np.ascontiguousarray(est_b1.reshape(V, 1))
    if has_b2:
        shared["B2"] = np.ascontiguousarray(est_b2.astype(bf16))  # [K, V]
    if has_pb1:
        shared["PB1"] = np.ascontiguousarray(pl_b1.reshape(128, 1))
    if has_pb2:
        shared["PB2"] = np.ascontiguousarray(pl_b2.reshape(64, 1))

    in_maps = []
    for r in range(NCORES):
        cols = slice(r * Bc, (r + 1) * Bc)
        m = dict(shared)
        m["inT"] = np.ascontiguousarray(inT[:, cols])
        m["xTb"] = np.ascontiguousarray(xTb[:, cols])
        m["smb"] = np.ascontiguousarray(smb[:, cols])
        in_maps.append(m)

    import os
    trace = bool(int(os.environ.get("CFG_KERNEL_TRACE", "0")))
    res = run_bass_kernel_spmd(nc, in_maps, core_ids=list(range(NCORES)),
                               trace=trace)
    if trace:
        kernel.last_result = res

    final_cf = np.empty((B, V), np.float32)
    effects = np.empty((B, V), np.float32)
    logits = np.empty(B, np.float32)
    for r in range(NCORES):
        rows = slice(r * Bc, (r + 1) * Bc)
        final_cf[rows] = res.results[r]["outC"].T
        effects[rows] = res.results[r]["outE"].T
        logits[rows] = res.results[r]["outP"][0]

    plaus = 1.0 / (1.0 + np.exp(-(logits + pl_b3.reshape(-1)[0])))
    d = final_cf - x
    impact = np.sqrt(np.einsum("ij,ij->i", d, d, dtype=np.float64,
                               optimize=True))
    return (final_cf, effects,
            plaus.astype(np.float32).reshape(B, 1),
            impact.astype(np.float32).reshape(B, 1))


# revision 21
# speedup vs baseline: 1.2584x; 1.2584x over previous
# Trainium2 Bass kernel for nn_CounterfactualGenerator (gnn_message_passing).
#
# Strategy:
#   * Pure data parallel over the batch dim B across 8 NeuronCores.
#   * The sequential 3x2048-edge propagation scan is a LINEAR operator on each
#     row's V-vector: every edge update d[:,e] += d[:,c]*(s*0.1) is an
#     elementary matrix A_i = I + coef*E_{c,e}; the full scan is d0 @ (A_1...A_N).
#     We form M = prod A_i on the host in float64 (O(E*V) work, ~ms) and apply
#     it on-device as one [B,V]x[V,V] matmul.  To keep fp32 accuracy with bf16
#     matmuls we split M = I + Mc and compute cf = pre + pre@Mc (the identity
#     part stays exact fp32; only the correction sees bf16 rounding).
#   * Everything on device lives in "transposed" layout [feature, batch]:
#     all matmuls contract over SBUF partitions, batch streams along the free
#     dim (N=512 chunks, one PSUM bank per matmul).
#   * The plausibility input concat [x, intervened, cf] is algebraically
#     folded: intervened = x + delta (delta nonzero on the K intervened rows
#     only), so  (pl_W1x + pl_W1int)^T x + pl_W1int[ivars]^T delta
#     + pl_W1cf^T cf, with the combine done on the host and the delta term a
#     K=4 matmul. 'intervened' (bf16) is never materialized on device; the
#     fp32 intervened matrix (needed for pre = intervened + eff) is built on
#     the host and shipped instead of fp32 x.
#   * Host does: input transpose/shard, M precompute, final transpose back,
#     sigmoid (plaus) and the row-norm (impact) from returned final_cf.
import sys

sys.path.insert(0, "/opt/trn_rl_repo")

import numpy as np
import ml_dtypes

import concourse.bass as bass
import concourse.mybir as mybir
import concourse.tile as tile
from concourse import bacc
from concourse.bass_utils import run_bass_kernel_spmd

NCORES = 8
V = 256
K = 4
NB = 512  # matmul free-dim chunk (one PSUM bank)
PROP_SCALE = 0.1
INTERVENTION_STRENGTH = 1.0

F32 = mybir.dt.float32
BF16 = mybir.dt.bfloat16
bf16 = ml_dtypes.bfloat16

_KERNEL_CACHE = {}


def _build_nc(Bc, has_b1, has_b2, has_pb1, has_pb2):
    """Build the per-core Bass module. Bc = batch elements per core."""
    nchunk = Bc // NB
    nc = bacc.Bacc("TRN2", target_bir_lowering=False, debug=False, num_devices=NCORES)

    # ---- DRAM I/O ----
    inT = nc.dram_tensor("inT", [V, Bc], F32, kind="ExternalInput")  # intervened^T
    xTb = nc.dram_tensor("xTb", [V, Bc], BF16, kind="ExternalInput")  # x^T bf16
    # [0:K] = |iv - x_iv| (mag), [K:2K] = iv - x_iv (delta for plaus fold)
    smb = nc.dram_tensor("smb", [2 * K, Bc], BF16, kind="ExternalInput")
    w1t = nc.dram_tensor("W1T", [V, V], BF16, kind="ExternalInput")  # [vin, o]
    w2t = nc.dram_tensor("W2T", [V, V], BF16, kind="ExternalInput")  # [o, vout]
    mct = nc.dram_tensor("MC", [V, V], BF16, kind="ExternalInput")  # [vin, vout]
    selt = nc.dram_tensor("SEL", [K, V], BF16, kind="ExternalInput")
    pw1xt = nc.dram_tensor("PW1X", [V, 128], BF16, kind="ExternalInput")
    pw1ct = nc.dram_tensor("PW1C", [V, 128], BF16, kind="ExternalInput")
    pw1dt = nc.dram_tensor("PW1D", [K, 128], BF16, kind="ExternalInput")
    pw2t = nc.dram_tensor("PW2T", [128, 64], BF16, kind="ExternalInput")
    pw3t = nc.dram_tensor("PW3T", [64, 1], BF16, kind="ExternalInput")
    if has_b1:
        b1t = nc.dram_tensor("B1", [V, 1], F32, kind="ExternalInput")
    if has_b2:
        b2t = nc.dram_tensor("B2", [K, V], BF16, kind="ExternalInput")
    if has_pb1:
        pb1t = nc.dram_tensor("PB1", [128, 1], F32, kind="ExternalInput")
    if has_pb2:
        pb2t = nc.dram_tensor("PB2", [64, 1], F32, kind="ExternalInput")

    outC = nc.dram_tensor("outC", [V, Bc], F32, kind="ExternalOutput")  # final_cf^T
    outE = nc.dram_tensor("outE", [V, Bc], F32, kind="ExternalOutput")  # effects^T
    outP = nc.dram_tensor("outP", [1, Bc], F32, kind="ExternalOutput")  # plaus logits

    # chunk-pair views: [part, vchunk, batch]
    inTv = inT.rearrange("(c p) b -> p c b", p=128)
    xTbv = xTb.rearrange("(c p) b -> p c b", p=128)
    outCv = outC.rearrange("(c p) b -> p c b", p=128)
    outEv = outE.rearrange("(c p) b -> p c b", p=128)

    AF = mybir.ActivationFunctionType

    with tile.TileContext(nc) as tc:
        with (
            tc.tile_pool(name="consts", bufs=1) as consts,
            tc.tile_pool(name="work", bufs=4) as work,
            tc.tile_pool(name="psum", bufs=1, space="PSUM") as psum,
        ):
            # ---- load replicated weights once ----
            w1 = [consts.tile([128, V], BF16, name=f"w1_{c}") for c in range(2)]
            w2 = [consts.tile([128, V], BF16, name=f"w2_{c}") for c in range(2)]
            mc = [consts.tile([128, V], BF16, name=f"mc_{c}") for c in range(2)]
            for c in range(2):
                nc.sync.dma_start(out=w1[c], in_=w1t[c * 128 : (c + 1) * 128, :])
                nc.sync.dma_start(out=w2[c], in_=w2t[c * 128 : (c + 1) * 128, :])
                nc.sync.dma_start(out=mc[c], in_=mct[c * 128 : (c + 1) * 128, :])
            pw1x = [consts.tile([128, 128], BF16, name=f"pw1x_{c}") for c in range(2)]
            pw1c = [consts.tile([128, 128], BF16, name=f"pw1c_{c}") for c in range(2)]
            for c in range(2):
                nc.sync.dma_start(out=pw1x[c], in_=pw1xt[c * 128 : (c + 1) * 128, :])
                nc.sync.dma_start(out=pw1c[c], in_=pw1ct[c * 128 : (c + 1) * 128, :])
            sel = consts.tile([K, V], BF16, name="sel")
            nc.sync.dma_start(out=sel, in_=selt[:, :])
            pw1d = consts.tile([K, 128], BF16, name="pw1d")
            nc.sync.dma_start(out=pw1d, in_=pw1dt[:, :])
            pw2 = consts.tile([128, 64], BF16, name="pw2")
            nc.sync.dma_start(out=pw2, in_=pw2t[:, :])
            pw3 = consts.tile([64, 1], BF16, name="pw3")
            nc.sync.dma_start(out=pw3, in_=pw3t[:, :])
            b1 = None
            if has_b1:
                b1 = [consts.tile([128, 1], F32, name=f"b1_{c}") for c in range(2)]
                for c in range(2):
                    nc.sync.dma_start(out=b1[c], in_=b1t[c * 128 : (c + 1) * 128, :])
            b2 = None
            if has_b2:
                b2 = consts.tile([K, V], BF16, name="b2")
                nc.sync.dma_start(out=b2, in_=b2t[:, :])
            pb1 = None
            if has_pb1:
                pb1 = consts.tile([128, 1], F32, name="pb1")
                nc.sync.dma_start(out=pb1, in_=pb1t[:, :])
            pb2 = None
            if has_pb2:
                pb2 = consts.tile([64, 1], F32, name="pb2")
                nc.sync.dma_start(out=pb2, in_=pb2t[:, :])

            # ---- hoisted mag broadcast: mf_all[p, c, b] = mag[(c*128+p)//64, b]
            # via a single DMA with a partition-broadcast access pattern
            dvall = consts.tile([K, Bc], BF16, name="dvall")
            nc.sync.dma_start(out=dvall, in_=smb[K : 2 * K, :])
            xball = consts.tile([128, 2, Bc], BF16, name="xball")
            nc.sync.dma_start(out=xball, in_=xTbv[:, :, :])
            plfall = consts.tile([1, Bc], F32, name="plfall")
            mf_all = consts.tile([128, 2, Bc], BF16, name="mf_all")
            magall = consts.tile([K, Bc], BF16, name="magall")
            nc.sync.dma_start(out=magall, in_=smb[0:K, :])
            for j in range(nchunk):
                colsj = slice(j * NB, (j + 1) * NB)
                for c in range(2):
                    mfp = psum.tile([128, NB], F32, name="mfp", tag="ps2", bufs=2)
                    nc.tensor.matmul(mfp, sel[:, c * 128 : (c + 1) * 128],
                                     magall[:, colsj], start=True, stop=True)
                    if c == 0:
                        nc.scalar.copy(mf_all[:, c, colsj], mfp)
                    else:
                        nc.vector.tensor_copy(mf_all[:, c, colsj], mfp)

            # ---- per-chunk pipeline ----
            for j in range(nchunk):
                cols = slice(j * NB, (j + 1) * NB)

                inf = work.tile([128, 2, NB], F32, name="inf", tag="inf")
                nc.sync.dma_start(out=inf, in_=inTv[:, :, cols])
                xb = xball[:, :, cols]
                dvb = dvall[:, cols]

                # h^T = relu(W1cat @ x + b1)  [256, NB]
                hp = psum.tile([128, 2, NB], F32, name="hp", tag="ps1", bufs=3)
                for c in range(2):
                    nc.tensor.matmul(hp[:, c, :],
                                     w1[0][:, c * 128 : (c + 1) * 128],
                                     xb[:, 0, :], start=True, stop=False)
                    nc.tensor.matmul(hp[:, c, :],
                                     w1[1][:, c * 128 : (c + 1) * 128],
                                     xb[:, 1, :], start=False, stop=True)
                hr = work.tile([128, 2, NB], BF16, name="hr", tag="hr")
                if has_b1:
                    for c in range(2):
                        nc.scalar.activation(hr[:, c, :], hp[:, c, :], AF.Relu,
                                             bias=b1[c])
                else:
                    nc.scalar.activation(hr, hp, AF.Relu)
                # hs = h * mag_broadcast  (bf16 x bf16, single 2x-mode op)
                hs = work.tile([128, 2, NB], BF16, name="hs", tag="hs")
                nc.vector.tensor_mul(hs, hr, mf_all[:, :, cols])

                # eff^T = W2cat @ hs (+ b2-part)  [256, NB]
                ep = psum.tile([128, 2, NB], F32, name="ep", tag="ps1", bufs=3)
                for c in range(2):
                    nc.tensor.matmul(ep[:, c, :],
                                     w2[0][:, c * 128 : (c + 1) * 128],
                                     hs[:, 0, :], start=True, stop=False)
                    nc.tensor.matmul(ep[:, c, :],
                                     w2[1][:, c * 128 : (c + 1) * 128],
                                     hs[:, 1, :], start=False, stop=not has_b2)
                    if has_b2:
                        nc.tensor.matmul(ep[:, c, :],
                                         b2[:, c * 128 : (c + 1) * 128],
                                         magall[:, cols], start=False, stop=True)

                # pre = intervened + eff (fp32); host recovers eff = pre - intv
                pf = work.tile([128, 2, NB], F32, name="pf", tag="pf")
                nc.vector.tensor_add(pf, ep, inf)
                nc.scalar.dma_start(out=outEv[:, :, cols], in_=pf)
                pb = work.tile([128, 2, NB], BF16, name="pb", tag="pb")
                nc.vector.tensor_copy(pb, pf)

                # cf = pre + pre @ Mc
                cp = psum.tile([128, 2, NB], F32, name="cp", tag="ps1", bufs=3)
                for c in range(2):
                    nc.tensor.matmul(cp[:, c, :],
                                     mc[0][:, c * 128 : (c + 1) * 128],
                                     pb[:, 0, :], start=True, stop=False)
                    nc.tensor.matmul(cp[:, c, :],
                                     mc[1][:, c * 128 : (c + 1) * 128],
                                     pb[:, 1, :], start=False, stop=True)
                cf = work.tile([128, 2, NB], F32, name="cf", tag="cf")
                nc.vector.tensor_add(cf, pf, cp)
                nc.gpsimd.dma_start(out=outCv[:, :, cols], in_=cf)

                # plausibility MLP: pin = [x, intervened, cf] folded; the cf
                # block reads pre via PW1C := M @ pl_W1cf^T (host):
                # p1 = (W1x+W1int)^T x + W1int[ivars]^T dv + (M@W1cf)^T pre
                p1p = psum.tile([128, NB], F32, name="p1p", tag="ps2", bufs=2)
                nc.tensor.matmul(p1p, pw1x[0], xb[:, 0, :], start=True, stop=False)
                nc.tensor.matmul(p1p, pw1x[1], xb[:, 1, :], start=False, stop=False)
                nc.tensor.matmul(p1p, pw1d, dvb, start=False, stop=False)
                nc.tensor.matmul(p1p, pw1c[0], pb[:, 0, :], start=False, stop=False)
                nc.tensor.matmul(p1p, pw1c[1], pb[:, 1, :], start=False, stop=True)
                p1b = work.tile([128, NB], BF16, name="p1b", tag="p1b")
                nc.scalar.activation(p1b, p1p, AF.Relu,
                                     bias=pb1 if has_pb1 else 0.0)
                p2p = psum.tile([64, NB], F32, name="p2p", tag="ps2", bufs=2)
                nc.tensor.matmul(p2p, pw2, p1b, start=True, stop=True)
                p2b = work.tile([64, NB], BF16, name="p2b", tag="p2b")
                nc.scalar.activation(p2b, p2p, AF.Relu,
                                     bias=pb2 if has_pb2 else 0.0)
                plp = psum.tile([1, NB], F32, name="plp", tag="ps2", bufs=2)
                nc.tensor.matmul(plp, pw3, p2b, start=True, stop=True)
                nc.scalar.copy(plfall[:, cols], plp)

            nc.sync.dma_start(out=outP[:, :], in_=plfall)

    nc.compile()
    return nc


def _prep_propagation_matrix(cause_idx, effect_idx, strengths, n_rounds=3):
    M = np.eye(V, dtype=np.float64)
    coef = strengths.astype(np.float64) * PROP_SCALE
    c = cause_idx.astype(np.int64)
    e = effect_idx.astype(np.int64)
    for _ in range(n_rounds):
        for i in range(len(c)):
            M[:, e[i]] += coef[i] * M[:, c[i]]
    return M


def kernel(original_data, intervention_values, strengths,
           est_W1, est_b1, est_W2, est_b2,
           pl_W1, pl_b1, pl_W2, pl_b2, pl_W3, pl_b3,
           intervention_variables, cause_idx, effect_idx):
    x = np.ascontiguousarray(np.asarray(original_data, dtype=np.float32))
    iv = np.asarray(intervention_values, dtype=np.float32)
    ivars = np.asarray(intervention_variables, dtype=np.int64)
    B = x.shape[0]
    assert x.shape[1] == V and iv.shape[1] == K
    assert B % (NCORES * NB) == 0, "batch must divide 8*512"
    Bc = B // NCORES

    est_b1 = np.asarray(est_b1, np.float32)
    est_b2 = np.asarray(est_b2, np.float32)
    pl_b1 = np.asarray(pl_b1, np.float32)
    pl_b2 = np.asarray(pl_b2, np.float32)
    pl_b3 = np.asarray(pl_b3, np.float32)
    has_b1 = bool(np.any(est_b1 != 0))
    has_b2 = bool(np.any(est_b2 != 0))
    has_pb1 = bool(np.any(pl_b1 != 0))
    has_pb2 = bool(np.any(pl_b2 != 0))

    # ---- host precompute ----
    M = _prep_propagation_matrix(np.asarray(cause_idx), np.asarray(effect_idx),
                                 np.asarray(strengths, np.float32))
    Mc = (M - np.eye(V)).astype(np.float32)

    W1cat = np.asarray(est_W1, np.float32).reshape(K * 64, V)  # [o, vin]
    W1T = np.ascontiguousarray(W1cat.T).astype(bf16)  # [vin, o]
    W2T = np.ascontiguousarray(
        np.asarray(est_W2, np.float32).transpose(0, 2, 1).reshape(K * 64, V)
    ).astype(bf16)  # [o, vout]
    MC = Mc.astype(bf16)  # [vin, vout]
    SEL = np.zeros((K, V), np.float32)
    for k in range(K):
        SEL[k, k * 64:(k + 1) * 64] = 1.0
    SEL = SEL.astype(bf16)
    pl_W1 = np.asarray(pl_W1, np.float32)  # [128, 3V]
    PW1X = np.ascontiguousarray((pl_W1[:, 0:V] + pl_W1[:, V:2 * V]).T).astype(bf16)
    PW1C = np.ascontiguousarray(
        M.astype(np.float64) @ pl_W1[:, 2 * V:3 * V].T.astype(np.float64)
    ).astype(np.float32).astype(bf16)
    # delta rows of the intervened block, deduped (last write wins)
    PW1D = np.zeros((K, 128), np.float32)
    last = {}
    for k, vv in enumerate(ivars.tolist()):
        last[vv] = k
    for k, vv in enumerate(ivars.tolist()):
        if last[vv] == k:
            PW1D[k] = pl_W1[:, V + vv]
    PW1D = PW1D.astype(bf16)
    PW2T = np.ascontiguousarray(np.asarray(pl_W2, np.float32).T).astype(bf16)
    PW3T = np.ascontiguousarray(np.asarray(pl_W3, np.float32).T).astype(bf16)

    intv = x.copy()
    intv[:, ivars] = iv
    inT = np.ascontiguousarray(intv.T)  # [V, B] fp32
    xTb = np.ascontiguousarray(x.T).astype(bf16)
    x_iv = x[:, ivars]  # [B, K]
    dv = iv - x_iv  # [B, K]
    # dedupe delta contributions for the plaus fold (last write wins)
    dvz = dv.copy()
    for k, vv in enumerate(ivars.tolist()):
        if last[vv] != k:
            dvz[:, k] = 0.0
    smb = np.concatenate([np.abs(dv).T, dvz.T], axis=0).astype(bf16)  # [2K, B]

    key = (Bc, has_b1, has_b2, has_pb1, has_pb2)
    if key not in _KERNEL_CACHE:
        _KERNEL_CACHE[key] = _build_nc(*key)
    nc = _KERNEL_CACHE[key]

    shared = {
        "W1T": W1T, "W2T": W2T, "MC": MC, "SEL": SEL,
        "PW1X": PW1X, "PW1C": PW1C, "PW1D": PW1D,
        "PW2T": PW2T, "PW3T": PW3T,
    }
    if has_b1:
        shared["B1"] = np.ascontiguousarray(est_b1.reshape(V, 1))
    if has_b2:
        shared["B2"] = np.ascontiguousarray(est_b2.astype(bf16))  # [K, V]
    if has_pb1:
        shared["PB1"] = np.ascontiguousarray(pl_b1.reshape(128, 1))
    if has_pb2:
        shared["PB2"] = np.ascontiguousarray(pl_b2.reshape(64, 1))

    in_maps = []
    for r in range(NCORES):
        cols = slice(r * Bc, (r + 1) * Bc)
        m = dict(shared)
        m["inT"] = np.ascontiguousarray(inT[:, cols])
        m["xTb"] = np.ascontiguousarray(xTb[:, cols])
        m["smb"] = np.ascontiguousarray(smb[:, cols])
        in_maps.append(m)

    import os
    trace = bool(int(os.environ.get("CFG_KERNEL_TRACE", "0")))
    res = run_bass_kernel_spmd(nc, in_maps, core_ids=list(range(NCORES)),
                               trace=trace)
    if trace:
        kernel.last_result = res

    final_cf = np.empty((B, V), np.float32)
    effects = np.empty((B, V), np.float32)
    logits = np.empty(B, np.float32)
    for r in range(NCORES):
        rows = slice(r * Bc, (r + 1) * Bc)
        final_cf[rows] = res.results[r]["outC"].T
        effects[rows] = res.results[r]["outE"].T
        logits[rows] = res.results[r]["outP"][0]
    effects -= intv  # device ships pre = intv + eff

    plaus = 1.0 / (1.0 + np.exp(-(logits + pl_b3.reshape(-1)[0])))
    d = final_cf - x
    impact = np.sqrt(np.einsum("ij,ij->i", d, d, dtype=np.float64,
                               optimize=True))
    return (final_cf, effects,
            plaus.astype(np.float32).reshape(B, 1),
            impact.astype(np.float32).reshape(B, 1))
